# revision 6
# baseline (speedup 1.0000x reference)
"""CategoricalGCNEncoder on 8 Trainium2 NeuronCores (Bass/Tile).

Design ("v8"):
  - Nodes (dst) sharded across 8 cores; per-layer feature tables AllGathered
    in 4 window-chunks, each issued as soon as its windows' epilogues finish
    so the collective overlaps the producing phase (embed for table1, layer-1
    for table2). Only the used row prefix (HID / OUT cols) is transferred.
  - All matmuls in bf16 (single PE pass). PSUM stays f32.
  - Embedding + first matmul fused: h1 = sum_f onehot_f @ T_f with
    T_f = emb_f @ W1_f computed on device (f32) then cast to bf16; one-hot
    encodings uploaded from host in bf16, batched EGW windows per DMA.
  - GCN normalization folded into node scaling; self-loop term and biases are
    accumulated on the PE: identity matmul adds the node's own (scaled) row,
    a rank-1 matmul adds sqrt(deg) x b so the later dis scaling yields +b.
  - gamma1/beta1 are folded into W2 on the host (W2g = g1*W2, r2 = be1@W2
    added as a rank-1 term), so layer-1's LN needs no per-feature ops.
  - Edge phase: per-edge rows gathered with gpsimd.dma_gather (int16 idx,
    256B bf16 rows, 4 SWDGE queues).
  - Segment-sum: per dst-window (128 nodes) PSUM accumulation of
    matmul(lhsT=S_col[128x128] bf16, rhs=msg_col[128xF] bf16). S is built
    on-device by is_equal against a materialized iota; dstrel is uploaded
    pair-replicated so every operand is unit-stride (DVE 2x 16-bit mode).
  - LayerNorm: bn_stats/bn_aggr, then xn = (x - mu) * rstd fused in one
    vector scalar_tensor_tensor op.
  - Host packs nodes into windows (vector bin packing) so every (window,
    bucket) has exactly 4 columns of 128 edge slots; the node->slot
    permutation is undone on the host at the end.
"""

import numpy as np
import ml_dtypes

import concourse.bass as bass
import concourse.mybir as mybir
import concourse.tile as tile
from concourse import bacc
from concourse.bass_utils import run_bass_kernel_spmd

BF16 = ml_dtypes.bfloat16

# ---------------- problem constants (hardcoded; kernel must be self-contained)
N = 100000
E = 1600000
NF = 8
EMB = 16
IN_DIM = 128
HID = 64
OUT = 32
NCAT = 100
EPS = 1e-5

NCORE = 8
SH = N // NCORE            # 12500 nodes per core
P = 128
W = 104                    # windows per core
SLOTS = W * P              # 13312 slots per core (>= SH)
KQ = 4                     # columns per (window, bucket)
NQ = 4                     # src buckets == SWDGE queues
COLS = W * KQ              # columns per bucket stream (416)
TOTCOL = NQ * COLS         # total columns (1664)
TOTPOS = TOTCOL * P        # total edge slots (212992)
TBL = NCORE * SLOTS        # table rows (106496)
BUCK = TBL // NQ           # bucket size (26624) < 32768
GW = 4                     # windows per gather group
CAP_Q = KQ * P             # 512 edge slots per (w, q)
ROWC = 128                 # table row width (bf16) -> 256B rows for gather
EGW = 4                    # windows per onehot DMA batch
NCHUNK = 4                 # AllGather chunks
CW = W // NCHUNK           # windows per chunk (26)
CWP = CW * P               # bounce rows per chunk (3328)

f32 = mybir.dt.float32
bf16 = mybir.dt.bfloat16
i16 = mybir.dt.int16

_CACHE = {}


# ------------------------------------------------------------------ program
def build_program():
    nc = bacc.Bacc(None, target_bir_lowering=False, debug=False,
                   num_devices=NCORE, num_swdge_queues=NQ,
                   dynamic_dma_scratch_size=16384)
    with tile.TileContext(nc) as tc:
        _build(nc, tc)
    nc.compile()
    return nc


def _build(nc, tc):
    AF = mybir.ActivationFunctionType
    ALU = mybir.AluOpType

    from contextlib import ExitStack
    ctx = ExitStack()
    dram = ctx.enter_context(tc.tile_pool(name="dram", bufs=1, space="DRAM"))
    const = ctx.enter_context(tc.tile_pool(name="const", bufs=1))
    oh_pool = ctx.enter_context(tc.tile_pool(name="ohp", bufs=3))
    msg_pool = ctx.enter_context(tc.tile_pool(name="msgp", bufs=12))
    s_pool = ctx.enter_context(tc.tile_pool(name="sp", bufs=4))
    epi_pool = ctx.enter_context(tc.tile_pool(name="epip", bufs=4))
    psum_mm = ctx.enter_context(tc.tile_pool(name="psmm", bufs=3, space="PSUM"))
    psum_tr = ctx.enter_context(tc.tile_pool(name="pstr", bufs=2, space="PSUM"))
    psum_w2 = ctx.enter_context(tc.tile_pool(name="psw2", bufs=2, space="PSUM"))

    def din(name, shape, dtype=f32):
        return dram.tile(shape, dtype, kind="ExternalInput", name=name,
                         uniquify=False)

    # ---- inputs
    onehot = din("onehot", [W // EGW, NCAT, EGW * NF * P], bf16)
    idxs = din("idxs", [P, TOTPOS // 16], i16)
    dstrel2 = din("dstrel2", [P, TOTCOL * 2], bf16)   # w-major, pair-replicated
    iotar = din("iotar", [P, NQ * KQ * P], bf16)
    degin = din("deg", [P, W])
    embT = din("embT", [EMB, NF * NCAT])
    w1 = din("w1", [EMB, NF, HID])
    w2g = din("w2g", [HID, OUT], bf16)                # gamma1-folded W2
    sqdeg = din("sqdeg", [1, W * P], bf16)            # sqrt(deg) per slot
    b1row = din("b1row", [1, HID], bf16)
    b2row = din("b2row", [1, OUT], bf16)
    r2row = din("r2row", [1, OUT], bf16)              # beta1 @ W2
    onesrow = din("onesrow", [1, P], bf16)
    g2r = din("g2r", [P, OUT])
    be2r = din("be2r", [P, OUT])
    identin = din("ident", [P, P])
    identbin = din("identb", [P, P], bf16)

    outx = dram.tile([SLOTS, OUT], f32, kind="ExternalOutput", name="outx",
                     uniquify=False)

    bounce1 = dram.tile([SLOTS, ROWC], bf16)
    table1 = dram.tile([TBL, ROWC], bf16, addr_space="Shared")
    bounce2 = dram.tile([SLOTS, ROWC], bf16)
    table2 = dram.tile([TBL, ROWC], bf16, addr_space="Shared")

    # ---- static SBUF
    idx_sb = const.tile([P, TOTPOS // 16], i16)
    nc.sync.dma_start(out=idx_sb[:], in_=idxs[:])
    dstrel2_sb = const.tile([P, TOTCOL * 2], bf16)
    nc.sync.dma_start(out=dstrel2_sb[:], in_=dstrel2[:])
    iota_sb = const.tile([P, NQ * KQ * P], bf16)
    nc.sync.dma_start(out=iota_sb[:], in_=iotar[:])
    ident_sb = const.tile([P, P], f32)
    nc.sync.dma_start(out=ident_sb[:], in_=identin[:])
    identb_sb = const.tile([P, P], bf16)
    nc.sync.dma_start(out=identb_sb[:], in_=identbin[:])
    w1_sb = const.tile([EMB, NF, HID], f32)
    nc.sync.dma_start(out=w1_sb[:], in_=w1[:])
    w2_sb = const.tile([HID, OUT], bf16)
    nc.sync.dma_start(out=w2_sb[:], in_=w2g[:])
    embT_sb = const.tile([EMB, NF * NCAT], f32)
    nc.sync.dma_start(out=embT_sb[:], in_=embT[:])
    sq_sb = const.tile([1, W * P], bf16)
    nc.sync.dma_start(out=sq_sb[:], in_=sqdeg[:])
    b1_sb = const.tile([1, HID], bf16)
    nc.sync.dma_start(out=b1_sb[:], in_=b1row[:])
    b2_sb = const.tile([1, OUT], bf16)
    nc.sync.dma_start(out=b2_sb[:], in_=b2row[:])
    r2_sb = const.tile([1, OUT], bf16)
    nc.sync.dma_start(out=r2_sb[:], in_=r2row[:])
    ones_sb = const.tile([1, P], bf16)
    nc.sync.dma_start(out=ones_sb[:], in_=onesrow[:])
    g2_sb = const.tile([P, OUT], f32)
    nc.sync.dma_start(out=g2_sb[:], in_=g2r[:])
    be2_sb = const.tile([P, OUT], f32)
    nc.sync.dma_start(out=be2_sb[:], in_=be2r[:])
    eps_sb = const.tile([P, 1], f32)
    nc.vector.memset(eps_sb[:], EPS)

    # dis = 1/sqrt(deg)
    deg_sb = const.tile([P, W], f32)
    nc.sync.dma_start(out=deg_sb[:], in_=degin[:])
    dis_sb = const.tile([P, W], f32)
    nc.scalar.activation(out=dis_sb[:], in_=deg_sb[:], func=AF.Sqrt)
    nc.vector.reciprocal(out=dis_sb[:], in_=dis_sb[:])

    # ---- T_f = emb_f @ W1_f  -> T_sb [NCAT, NF, HID] bf16
    T_sb = const.tile([NCAT, NF, HID], bf16)
    for f in range(NF):
        pt = psum_mm.tile([NCAT, HID], f32, space="PSUM", tag="ps")
        nc.tensor.matmul(
            out=pt[:],
            lhsT=embT_sb[:, f * NCAT:(f + 1) * NCAT],
            rhs=w1_sb[:, f, :],
            start=True, stop=True,
        )
        nc.scalar.copy(out=T_sb[:, f, :], in_=pt[:])

    # stagings for the AllGather table rows; table cols >= fdim are never
    # read by the edge matmuls, so rows are bounced/gathered as prefixes.
    stag1 = const.tile([P, W, HID], bf16)
    stag2 = const.tile([P, W, OUT], bf16)

    def bounce_chunk(stag, fdim, bounce, c):
        lo, hi = c * CW, (c + 1) * CW
        nc.sync.dma_start(
            out=bounce.rearrange("(w p) h -> p w h", p=P)[:, lo:hi, :fdim],
            in_=stag[:, lo:hi, :])

    def ag_full(bounce, table_, fdim):
        # Shared DRAM allows a single writer: one AllGather per table.
        # (Collective APs must be contiguous, so full 256B rows move.)
        nc.gpsimd.collective_compute(
            "AllGather", mybir.AluOpType.bypass,
            replica_groups=[list(range(NCORE))],
            ins=[bounce[:]], outs=[table_[:]],
        )

    # ---- embedding: stag1[p, w, :] = dis * sum_f onehot_f_w.T @ T_f
    for g in range(W // EGW):
        oh = oh_pool.tile([NCAT, EGW * NF * P], bf16, tag="oh")
        nc.sync.dma_start(out=oh[:], in_=onehot[g])
        for gi in range(EGW):
            w = g * EGW + gi
            pe = psum_mm.tile([P, HID], f32, space="PSUM", tag="ps")
            for f in range(NF):
                nc.tensor.matmul(
                    out=pe[:],
                    lhsT=oh[:, (gi * NF + f) * P:(gi * NF + f + 1) * P],
                    rhs=T_sb[:, f, :],
                    start=(f == 0), stop=(f == NF - 1),
                )
            nc.scalar.activation(out=stag1[:, w, :], in_=pe[:], func=AF.Copy,
                                 scale=dis_sb[:, w:w + 1])
            if (w + 1) % CW == 0:
                bounce_chunk(stag1, HID, bounce1, w // CW)
    ag_full(bounce1, table1, HID)

    def build_s(w, engine):
        """S[p, (q,c), j] = (dstrel[p, w, q, c] == j), bf16, unit-stride."""
        s = s_pool.tile([P, NQ * KQ, P], bf16, tag="s")
        engine.tensor_tensor(
            out=s.rearrange("p qc (jh r) -> p qc jh r", r=2),
            in0=iota_sb.rearrange("p (qc jh r) -> p qc jh r", qc=NQ * KQ, r=2),
            in1=dstrel2_sb.rearrange("p (w qc r) -> p w qc r", w=W, r=2)
                [:, w, :, :]
                .rearrange("p qc (jh r) -> p qc jh r", jh=1)
                .to_broadcast([P, NQ * KQ, P // 2, 2]),
            op=ALU.is_equal,
        )
        return s

    def edge_layer(table, fdim, epilogue, post_window=None):
        """Gather+segment-sum over all edges; call epilogue(w, psum_tile)."""
        ngrp = W // GW
        for g in range(ngrp):
            msgs = []
            for q in range(NQ):
                m = msg_pool.tile([P, GW * KQ, ROWC], bf16, tag="msg")
                c0 = (q * W + g * GW) * KQ          # first column of chunk
                nc.gpsimd.dma_gather(
                    m[:], table[BUCK * q:BUCK * (q + 1), :],
                    idx_sb[:, c0 * 8:(c0 + GW * KQ) * 8],
                    num_idxs=GW * KQ * P, num_idxs_reg=GW * KQ * P,
                    elem_size=ROWC, single_packet=False, queue_num=q,
                )
                msgs.append(m)
            for wi in range(GW):
                w = g * GW + wi
                s = build_s(w, nc.vector)
                pt = psum_mm.tile([P, fdim], f32, space="PSUM", tag="ps")
                k = 0
                for q in range(NQ):
                    for c in range(KQ):
                        nc.tensor.matmul(
                            out=pt[:],
                            lhsT=s[:, q * KQ + c, :],
                            rhs=msgs[q][:, wi * KQ + c, :fdim],
                            start=(k == 0), stop=False,
                        )
                        k += 1
                epilogue(w, pt)
                if post_window is not None:
                    post_window(w)

    def ln_xn(x, fdim, tag):
        """LN stats of x [P, fdim] f32 -> xn = (x - mu) * rstd (epi tile)."""
        stats = epi_pool.tile([P, 1, 6], f32, tag=tag + "st")
        mv = epi_pool.tile([P, 2], f32, tag=tag + "mv")
        nc.vector.bn_stats(out=stats[:, 0, :], in_=x[:])
        nc.vector.bn_aggr(out=mv[:], in_=stats[:])
        rstd = epi_pool.tile([P, 1], f32, tag=tag + "rs")
        nc.scalar.activation(out=rstd[:], in_=mv[:, 1:2], func=AF.Sqrt,
                             bias=eps_sb[:], scale=1.0)
        nc.vector.reciprocal(out=rstd[:], in_=rstd[:])
        xn = epi_pool.tile([P, fdim], f32, tag=tag + "xn")
        nc.vector.scalar_tensor_tensor(
            out=xn[:], in0=x[:], scalar=mv[:, 0:1],
            in1=rstd[:].to_broadcast([P, fdim]),
            op0=ALU.subtract, op1=ALU.mult)
        return xn

    def epi1(w, pt):
        # finish PSUM: += self row + sqrt(deg) x b1 (so dis scaling gives +b1)
        nc.tensor.matmul(out=pt[:], lhsT=identb_sb[:], rhs=stag1[:, w, :],
                         start=False, stop=False)
        nc.tensor.matmul(out=pt[:], lhsT=sq_sb[0:1, w * P:(w + 1) * P],
                         rhs=b1_sb[:], start=False, stop=True)
        # x = relu(dis * psum)
        x = epi_pool.tile([P, HID], f32, tag="x1")
        nc.scalar.activation(out=x[:], in_=pt[:], func=AF.Relu,
                             scale=dis_sb[:, w:w + 1])
        xn = ln_xn(x, HID, "a")
        # h2 = dis * (xn @ W2g + 1 x r2): transpose xn then matmul (bf16)
        ptr = psum_tr.tile([HID, P], f32, space="PSUM", tag="tr")
        nc.tensor.transpose(out=ptr[:], in_=xn[:], identity=ident_sb[:])
        xnT = epi_pool.tile([HID, P], bf16, tag="xnT")
        nc.scalar.copy(out=xnT[:], in_=ptr[:])
        pw2 = psum_w2.tile([P, OUT], f32, space="PSUM", tag="w2")
        nc.tensor.matmul(out=pw2[:], lhsT=xnT[:], rhs=w2_sb[:],
                         start=True, stop=False)
        nc.tensor.matmul(out=pw2[:], lhsT=ones_sb[:], rhs=r2_sb[:],
                         start=False, stop=True)
        nc.scalar.activation(out=stag2[:, w, :], in_=pw2[:],
                             func=AF.Copy, scale=dis_sb[:, w:w + 1])

    final = const.tile([P, W, OUT], f32)

    def epi2(w, pt):
        nc.tensor.matmul(out=pt[:], lhsT=identb_sb[:], rhs=stag2[:, w, :],
                         start=False, stop=False)
        nc.tensor.matmul(out=pt[:], lhsT=sq_sb[0:1, w * P:(w + 1) * P],
                         rhs=b2_sb[:], start=False, stop=True)
        x = epi_pool.tile([P, OUT], f32, tag="x2")
        nc.scalar.activation(out=x[:], in_=pt[:], func=AF.Copy,
                             scale=dis_sb[:, w:w + 1])
        xn = ln_xn(x, OUT, "b")
        y = epi_pool.tile([P, OUT], f32, tag="y2")
        nc.vector.tensor_tensor(out=y[:], in0=xn[:], in1=g2_sb[:],
                                op=ALU.mult)
        nc.vector.tensor_add(out=final[:, w, :], in0=y[:], in1=be2_sb[:])
        if (w + 1) % CW == 0:
            nc.sync.dma_start(
                out=outx.rearrange("(w p) o -> p w o", p=P)
                [:, (w // CW) * CW:(w // CW + 1) * CW, :],
                in_=final[:, (w // CW) * CW:(w // CW + 1) * CW, :])

    # ---- layer 1
    def post1(w):
        # stage layer-2 bounce chunks as their windows' epilogues finish
        if (w + 1) % CW == 0:
            bounce_chunk(stag2, OUT, bounce2, w // CW)

    edge_layer(table1, HID, epi1, post_window=post1)
    ag_full(bounce2, table2, OUT)

    # ---- layer 2
    edge_layer(table2, OUT, epi2)
    ctx.close()


# ------------------------------------------------------------------ host prep
def _pack_core(dloc, q_of_edge):
    """Assign local nodes to (window, slot) with per-(w,q) capacity CAP_Q and
    <=P nodes per window.  Returns win[SH], pslot[SH]."""
    cnt = np.zeros((SH, NQ), np.int64)
    np.add.at(cnt, (dloc, q_of_edge), 1)
    tot = cnt.sum(1)
    order = np.argsort(-tot, kind="stable")
    fills = np.zeros((W, NQ), np.int64)
    counts = np.zeros(W, np.int64)
    win = np.zeros(SH, np.int64)
    for n in order:
        c = cnt[n]
        ok = (counts < P) & np.all(fills + c <= CAP_Q, axis=1)
        if not ok.any():
            raise RuntimeError("window packing failed")
        load = np.where(ok[:, None], fills + c, 1 << 30).max(axis=1)
        wsel = int(np.argmin(load))
        win[n] = wsel
        fills[wsel] += c
        counts[wsel] += 1
    pslot = np.zeros(SH, np.int64)
    for wsel in range(W):
        nodes = np.nonzero(win == wsel)[0]
        pslot[nodes] = np.arange(len(nodes))
    return win, pslot


def _host_prep(x_cat, edge_index, emb_tables, W1, b1, W2, b2,
               gamma1, beta1, gamma2, beta2):
    src = np.asarray(edge_index[0], np.int64)
    dst = np.asarray(edge_index[1], np.int64)
    deg = np.bincount(dst, minlength=N).astype(np.float64) + 1.0

    core_of = np.arange(N) // SH
    wins = np.zeros(N, np.int64)
    pslots = np.zeros(N, np.int64)
    srcq = src // (2 * SH)  # bucket of an edge = pair-of-cores owning src
    for k in range(NCORE):
        m = (dst // SH) == k
        dloc = dst[m] - k * SH
        win, ps = _pack_core(dloc, srcq[m])
        wins[k * SH:(k + 1) * SH] = win
        pslots[k * SH:(k + 1) * SH] = ps
    slot_of = wins * P + pslots               # slot within owner core
    trow = core_of * SLOTS + slot_of          # global table row

    # host-side folded weights
    W1f = np.asarray(W1, np.float32)
    W2f = np.asarray(W2, np.float32)
    g1f = np.asarray(gamma1, np.float32)
    be1f = np.asarray(beta1, np.float32)
    W2g = (g1f[:, None] * W2f).astype(BF16)
    r2 = (be1f @ W2f).astype(np.float32)

    in_maps = []
    perm_slots = []
    for k in range(NCORE):
        m = (dst // SH) == k
        es, ed = src[m], dst[m] - k * SH
        ew = wins[ed + k * SH]
        ep = pslots[ed + k * SH]
        eq = trow[es] // BUCK
        # stream position: per (q, w) block of CAP_Q slots, fill in order
        gkey = eq * W + ew
        order = np.argsort(gkey, kind="stable")
        gsort = gkey[order]
        start = np.searchsorted(gsort, np.arange(NQ * W))
        rank = np.arange(len(gsort)) - start[gsort]
        assert (rank < CAP_Q).all()
        pos = gsort * CAP_Q + rank
        idx16 = np.zeros(TOTPOS, np.int16)
        drel = np.full(TOTPOS, -1.0, np.float32)
        idx16[pos] = (trow[es][order] - eq[order] * BUCK).astype(np.int16)
        drel[pos] = ep[order].astype(np.float32)
        # wrap idx: j -> [j%16, j//16], replicate x8 partition groups
        idxw = np.tile(idx16.reshape(-1, 16).T, (8, 1))
        # dstrel, one value per (column, partition): [P, q, w, c] ->
        # reorder to w-major [P, w, q, c] and pair-replicate for unit stride
        drelw = np.ascontiguousarray(drel.reshape(-1, P).T)    # [P, (q w c)]
        drelw = drelw.reshape(P, NQ, W, KQ).transpose(0, 2, 1, 3)  # [P,w,q,c]
        drel2 = np.repeat(drelw.reshape(P, TOTCOL, 1), 2, axis=2)
        drel2 = np.ascontiguousarray(drel2.reshape(P, TOTCOL * 2)).astype(BF16)

        # onehot [W//EGW, NCAT, EGW*NF*P] bf16 for this core's slots
        oh = np.zeros((W, NCAT, NF, P), BF16)
        sl = slot_of[k * SH:(k + 1) * SH]
        xc = np.asarray(x_cat[k * SH:(k + 1) * SH], np.int64)
        wv = sl // P
        pv = sl % P
        for f in range(NF):
            oh[wv, xc[:, f], f, pv] = 1.0
        oh = np.ascontiguousarray(
            oh.reshape(W // EGW, EGW, NCAT, NF * P).transpose(0, 2, 1, 3)
            .reshape(W // EGW, NCAT, EGW * NF * P))

        degs = np.ones(SLOTS, np.float32)
        degs[sl] = deg[k * SH:(k + 1) * SH]
        degw = np.ascontiguousarray(degs.reshape(W, P).T)

        embT = np.ascontiguousarray(
            np.asarray(emb_tables, np.float32).transpose(2, 0, 1)
            .reshape(EMB, NF * NCAT))

        rep = lambda v, d: np.broadcast_to(
            np.asarray(v, np.float32).reshape(1, d), (P, d)).copy()

        iota_rep = np.broadcast_to(
            np.arange(P, dtype=np.float32), (P, NQ * KQ, P)).reshape(
            P, NQ * KQ * P).astype(BF16).copy()

        in_maps.append({
            "onehot": oh,
            "idxs": idxw,
            "dstrel2": drel2,
            "iotar": iota_rep,
            "deg": degw,
            "embT": embT,
            "w1": np.ascontiguousarray(W1f.reshape(NF, EMB, HID).transpose(1, 0, 2)),
            "w2g": W2g,
            "sqdeg": np.sqrt(degs).astype(BF16).reshape(1, SLOTS),
            "b1row": np.asarray(b1, np.float32).astype(BF16).reshape(1, HID),
            "b2row": np.asarray(b2, np.float32).astype(BF16).reshape(1, OUT),
            "r2row": r2.astype(BF16).reshape(1, OUT),
            "onesrow": np.ones((1, P), BF16),
            "g2r": rep(gamma2, OUT), "be2r": rep(beta2, OUT),
            "ident": np.eye(P, dtype=np.float32),
            "identb": np.eye(P, dtype=np.float32).astype(BF16),
        })
        perm_slots.append(sl)
    return in_maps, perm_slots


# ------------------------------------------------------------------ entry
def kernel(x_cat, edge_index, emb_tables, W1, b1, W2, b2,
           gamma1, beta1, gamma2, beta2, _res_hook=None):
    if "nc" not in _CACHE:
        _CACHE["nc"] = build_program()
    nc = _CACHE["nc"]
    in_maps, perm_slots = _host_prep(
        np.asarray(x_cat), np.asarray(edge_index), np.asarray(emb_tables),
        np.asarray(W1), np.asarray(b1), np.asarray(W2), np.asarray(b2),
        np.asarray(gamma1), np.asarray(beta1), np.asarray(gamma2),
        np.asarray(beta2))
    res = run_bass_kernel_spmd(nc, in_maps, list(range(NCORE)),
                               **(_res_hook or {}))
    out = np.empty((N, OUT), np.float32)
    for k in range(NCORE):
        full = res.results[k]["outx"]        # [SLOTS, OUT] slot-ordered
        out[k * SH:(k + 1) * SH] = full[perm_slots[k]]
    if _res_hook is not None:
        _res_hook["result"] = res
    return out
